# revision 39
# baseline (speedup 1.0000x reference)
"""Trainium2 Bass kernel for nn_DZSpecimenClfToy (v6).

Reference computation (per batch item b, B=8, one NeuronCore each):
  1. tv = bilinear_resize(topview[b], (3,64,64) -> (3,4,4))
  2. coords = sigmoid(tv.flat @ W1.T + b1).reshape(N,2)       # N=4096
  3. tl = coords*2043; 5x5x3 bilinear support per patch
  4. out[b] = bilinear_crops.flat @ W2.T + b2                 # [2]

Sharding: data-parallel over batch across 8 cores; weights replicated.

Host re-lays the search view as a cell table svc[r*2048+c] = 16 bf16
(rows r..r+4 of column c, 15 values + pad), so a patch at (r0,c0) is ONE
contiguous 79-bf16 run at cell index r0*2048+c0 (< 2^23: float magic
rounding gives the exact int index; no div/mod needed).

The HW indirect DMA supports one offset per partition per instruction
(verified: multi-offset tables generate garbage descriptors), so the
gather is 32 x [128 offsets] instructions serialized on the GpSimd Q7
(~1.4us each) - the dominant wall.  v6 minimizes everything around it
(~79.7us -> ~69.5us):

 - group 0 is small (6 patches/partition) and its coords chain runs with
   nothing fat interleaved, so the first gather issues ~14.8us instead
   of ~22us; the remaining 26 patch columns' coords run in two blocks
   floor-scheduled at 16.5/21us, and every combine group is floored past
   ~28us - without the floors the static scheduler slots fat coord muls
   or DMA-sem-waiting combine ops ahead of ready ops on the in-order
   Vector pipe, stalling the gather-critical chain (measured +4..11us).
 - b1 is folded into W1 as a 49th column (flat gets a constant 1.0).
 - the 64x64->4x4 resize runs on 3 partitions (one per channel) with a
   mask-built block-diagonal, then a 3-partition PE f32 matmul
   broadcasts flat (+the 1.0 lane) to all partitions.
 - W1 is loaded in three slices across both DMA rings so every coords
   block's slice is resident by ~10.5us.
 - group sizes descend [6,12,8,4,2]: the last gather's combine is tiny,
   so the post-gather tail is ~7us instead of ~12us.

Known dead ends (measured on HW): tensor_tensor_reduce hangs the device;
InstDMAGatherAnt (one instruction per 4096 descriptors, ~2.4us) works
but its int16 indices cannot address the 4.19M-cell table; gpsimd
tensor_scalar ops fail walrus ("engine check failed (Pool)").

floor() is computed as round_half_even(x - 0.5) in one fused op; the
half-integer cases land on a neighbouring cell with fraction 0/1, which
bilinear interpolation maps to the same value.  Fractions are produced
NEGATED (one fused op) and the combine uses S0-S1 / T0-T1 differences
to compensate.

Precision: coords matvec in f32 (bf16 shifts patches ~0.1px: fails);
patch data / combine / W2 in bf16 (~5e-3 rel err vs 2e-2 budget).
"""
import functools
from contextlib import ExitStack

import numpy as np
import ml_dtypes

import concourse.bass as bass
import concourse.tile as tile
from concourse import bacc, mybir
import concourse.bass_utils as bass_utils
from concourse.bass import IndirectOffsetOnAxis

F32 = mybir.dt.float32
BF16 = mybir.dt.bfloat16
I32 = mybir.dt.int32
ALU = mybir.AluOpType
ACT = mybir.ActivationFunctionType
AX = mybir.AxisListType

B = 8          # batch == number of cores
H = W = 2048   # search view height/width
N = 4096       # patches per item
PS = 4         # patch size
NCLS = 2       # classes
P = 128        # partitions
TPP = N // P   # patches per partition = 32

R = H - PS     # 2044 rows of 5-row cells (r0 in [0, 2043])
CELL = 16      # bf16 per (row, col) cell: 5 rows x 3 ch + 1 pad
SEG = 79       # gathered bf16 per patch: max offset 4*16+4*3+2 = 78
SEGP = 80      # SBUF stride per patch segment
MAGIC = 8388608.0      # 2**23
MAGICH = MAGIC - 0.5              # exact in f32 (below 2^23)
BIASF = MAGIC + 2.0               # r0b = BIASF + floor(tl)
C1 = BIASF * float(W)             # 2^34 + 2^12, exact in f32
NP_BF16 = ml_dtypes.bfloat16

CHUNKS = [6, 12, 8, 4, 2]  # patches per partition per gather/combine group
assert sum(CHUNKS) == TPP
NG = len(CHUNKS)
T0 = CHUNKS[0]
COORDS_B1 = 12             # second coords block size (g0 is the first)


def build_program(num_devices: int, svh: int, svw: int):
    pad = float(svh - 1 - PS)  # 2043
    assert svh == H and svw == W, (svh, svw)

    nc = bacc.Bacc("TRN2", target_bir_lowering=False, debug=False,
                   enable_asserts=False, num_devices=num_devices,
                   enable_partition_id=False)

    tvs = nc.dram_tensor("tvs", [3, 512], F32, kind="ExternalInput").ap()
    msk = nc.dram_tensor("msk", [3, 48], F32, kind="ExternalInput").ap()
    svc = nc.dram_tensor("svc", [R * W, CELL], BF16, kind="ExternalInput").ap()
    w1 = nc.dram_tensor("W1k", [P, 64 * 49], F32, kind="ExternalInput").ap()
    w2 = nc.dram_tensor("W2k", [P, NCLS * 1536], BF16, kind="ExternalInput").ap()
    b2 = nc.dram_tensor("b2k", [1, NCLS], F32, kind="ExternalInput").ap()
    out = nc.dram_tensor("out", [1, NCLS], F32, kind="ExternalOutput").ap()

    with tile.TileContext(nc) as tc:
        with ExitStack() as ctx:
            pool = ctx.enter_context(tc.tile_pool(name="main", bufs=1))
            ppool = ctx.enter_context(tc.tile_pool(name="ps", bufs=1,
                                                   space="PSUM"))

            # Everything through group 0's gathers at priority 0 so the
            # Tile scheduler keeps the head chain tight.
            prio = tc.high_priority()
            prio.__enter__()

            # ---- input DMAs.  scalar ring: tvs (gates the head), W2
            # (bulky, needed ~17us in), b2.  sync ring: W1 in two slices
            # (group-0 lanes first).
            Asb = pool.tile([3, 512], F32)
            nc.scalar.dma_start(Asb[:], tvs)
            msksb = pool.tile([3, 48], F32)
            nc.sync.dma_start(msksb[:], msk)
            # W1 split across both rings so every coords block's slice is
            # resident by ~10.5us (a late slice makes the scheduler park a
            # DMA-wait ahead of ready ops on the in-order Vector pipe).
            W1sb = pool.tile([P, 64 * 49], F32)
            ja, jb = 2 * T0 * 49, 2 * (T0 + COORDS_B1) * 49
            nc.sync.dma_start(W1sb[:, 0:ja], w1[:, 0:ja])
            nc.scalar.dma_start(W1sb[:, ja:jb], w1[:, ja:jb])
            nc.sync.dma_start(W1sb[:, jb:], w1[:, jb:])
            W2sb = pool.tile([P, NCLS * 1536], BF16)
            nc.scalar.dma_start(W2sb[:], w2)
            b2sb = pool.tile([1, NCLS], F32)
            nc.scalar.dma_start(b2sb[:], b2)

            # ---- topview 64x64 -> 4x4 resize on 3 partitions (1/channel).
            # Host pre-picked the 8 rows {16i+7, 16i+8}; row pairs then col
            # pairs {16j+7, 16j+8} are averaged (x0.25 folded into W1k).
            A4 = Asb[:].rearrange("p (i pair w) -> p i pair w", i=4, pair=2)
            V = pool.tile([3, 256], F32)
            nc.vector.tensor_add(V[:].rearrange("p (i w) -> p i w", i=4),
                                 A4[:, :, 0, :], A4[:, :, 1, :])
            V4 = V[:].rearrange("p (i j s) -> p i j s", i=4, j=4)
            flat3 = pool.tile([3, 16], F32)
            nc.vector.tensor_add(flat3[:].rearrange("p (i j) -> p i j", i=4),
                                 V4[:, :, :, 7], V4[:, :, :, 8])
            # block-diagonal [3, 49] (channel c in cols 16c..16c+15, col 48
            # = 1.0 for the folded b1), then PE broadcast to all partitions.
            bd = pool.tile([3, 49], F32)
            nc.vector.tensor_mul(
                bd[:, 0:48].rearrange("p (g e) -> p g e", g=3),
                flat3[:].unsqueeze(1).to_broadcast((3, 3, 16)),
                msksb[:].rearrange("p (g e) -> p g e", g=3))
            nc.vector.memset(bd[:, 48:49], 0.0)
            nc.vector.memset(bd[0:1, 48:49], 1.0)
            ones3 = pool.tile([3, P], F32)
            nc.vector.memset(ones3[:], 1.0)
            flatb = ppool.tile([P, 49], F32)
            nc.tensor.matmul(out=flatb[:], lhsT=ones3[:], rhs=bd[:],
                             start=True, stop=True)

            ones = pool.tile([P, 1], F32)
            nc.vector.memset(ones[:], 1.0)

            # ---- coords + gather indices --------------------------------
            mul1 = pool.tile([P, 64 * 49], F32)
            pre = pool.tile([P, 64], F32)
            sg = pool.tile([P, 64], F32)
            tl = pool.tile([P, 64], F32)
            r0b = pool.tile([P, 64], F32)   # 2^23 + floor(tl)
            nf = pool.tile([P, 64], F32)    # -fraction
            t1 = pool.tile([P, TPP], F32)
            idxm = pool.tile([P, TPP], F32)
            idxi = pool.tile([P, TPP], I32)
            frx = pool.tile([P, TPP * 12], BF16)   # -fr expanded x12
            fcx = pool.tile([P, TPP * 12], BF16)   # -fc expanded x12

            def coords_block(tbase, TPC, floor_ms=None):
                """matvec+sigmoid+index math for patch cols [tbase, tbase+TPC).

                floor_ms pins this block's matvec past group 0's chain so
                the scheduler cannot slot it ahead on the in-order Vector
                pipe (the rest of the block follows in program order).
                """
                E = nc.vector
                JC = 2 * TPC
                js = slice(2 * tbase, 2 * tbase + JC)
                ts = slice(tbase, tbase + TPC)
                m1v = mul1[:, 2 * tbase * 49:(2 * tbase + JC) * 49] \
                    .rearrange("p (j c) -> p j c", j=JC)
                with tc.tile_wait_until(floor_ms or 0, enable=floor_ms is not None):
                    nc.vector.tensor_mul(
                        m1v, W1sb[:, 2 * tbase * 49:(2 * tbase + JC) * 49]
                        .rearrange("p (j c) -> p j c", j=JC),
                        flatb[:].unsqueeze(1).to_broadcast((P, JC, 49)))
                nc.vector.reduce_sum(pre[:, js].unsqueeze(2), m1v, axis=AX.X)
                nc.scalar.activation(sg[:, js], pre[:, js], ACT.Sigmoid)
                E.tensor_scalar(tl[:, js], sg[:, js], pad, 2.0,
                                op0=ALU.mult, op1=ALU.add)
                E.tensor_scalar_add(r0b[:, js], tl[:, js], MAGICH)
                E.scalar_tensor_tensor(nf[:, js], r0b[:, js], MAGIC,
                                       tl[:, js],
                                       op0=ALU.subtract,
                                       op1=ALU.subtract)
                r0v = r0b[:, js].rearrange("p (t two) -> p t two", two=2)
                E.tensor_scalar(t1[:, ts], r0v[:, :, 0], float(W),
                                -C1, op0=ALU.mult, op1=ALU.add)
                E.scalar_tensor_tensor(idxm[:, ts], r0v[:, :, 1],
                                       2.0, t1[:, ts],
                                       op0=ALU.subtract, op1=ALU.add)
                E.tensor_single_scalar(idxi[:, ts],
                                       idxm[:, ts].bitcast(I32),
                                       0x007FFFFF, op=ALU.bitwise_and)
                nfv = nf[:, js].rearrange("p (t two) -> p t two", two=2)
                nc.vector.tensor_scalar_mul(
                    frx[:, tbase * 12:(tbase + TPC) * 12]
                    .rearrange("p (t e) -> p t e", t=TPC),
                    nfv[:, :, 0:1].to_broadcast((P, TPC, 12)), 1.0)
                nc.vector.tensor_scalar_mul(
                    fcx[:, tbase * 12:(tbase + TPC) * 12]
                    .rearrange("p (t e) -> p t e", t=TPC),
                    nfv[:, :, 1:2].to_broadcast((P, TPC, 12)), 1.0)

            Schunks = []
            for k, TPC in enumerate(CHUNKS):
                Sk = pool.tile([P, TPC * SEGP], BF16, tag=f"S{k}")
                Schunks.append(Sk)

            # Gather-critical chain at priority 0 in emission order:
            # g0 coords (small), g0 gathers, then the remaining coords in
            # two blocks, then the remaining gathers.  Combines (normal
            # priority) stream under the 45us gather phase.
            coords_block(0, T0)
            for t in range(T0):
                nc.gpsimd.indirect_dma_start(
                    out=Schunks[0][:, t * SEGP:t * SEGP + SEG],
                    out_offset=None,
                    in_=svc,
                    in_offset=IndirectOffsetOnAxis(
                        ap=idxi[:, t:t + 1], axis=0),
                )
            coords_block(T0, COORDS_B1, floor_ms=0.0165)
            coords_block(T0 + COORDS_B1, TPP - T0 - COORDS_B1, floor_ms=0.021)

            tbase = T0
            for k, TPC in enumerate(CHUNKS[1:], start=1):
                S = Schunks[k]
                for t in range(TPC):
                    tg = tbase + t
                    nc.gpsimd.indirect_dma_start(
                        out=S[:, t * SEGP:t * SEGP + SEG],
                        out_offset=None,
                        in_=svc,
                        in_offset=IndirectOffsetOnAxis(
                            ap=idxi[:, tg:tg + 1], axis=0),
                    )
                tbase += TPC
            prio.__exit__(None, None, None)

            # ---- per-group combine + classifier (overlaps later gathers)
            TPCmax = max(CHUNKS)
            D1 = pool.tile([P, TPCmax * 60], BF16)
            M1 = pool.tile([P, TPCmax * 60], BF16)
            T = pool.tile([P, TPCmax * 60], BF16)
            D2 = pool.tile([P, TPCmax * 48], BF16)
            M2 = pool.tile([P, TPCmax * 48], BF16)
            U = pool.tile([P, TPCmax * 48], BF16)
            Pm = pool.tile([P, NCLS * TPCmax * 48], BF16)
            r2all = pool.tile([P, NCLS * NG], F32)

            tbase = 0
            for k, TPC in enumerate(CHUNKS):
                S = Schunks[k]
                Sc = S[:].rearrange("p (t d e) -> p t d e", t=TPC, d=5)
                S0 = Sc[:, :, :, 0:12]
                S1 = Sc[:, :, :, 3:15]
                frb = frx[:, tbase * 12:(tbase + TPC) * 12] \
                    .rearrange("p (t e) -> p t e", t=TPC).unsqueeze(2) \
                    .to_broadcast((P, TPC, 5, 12))
                fcb = fcx[:, tbase * 12:(tbase + TPC) * 12] \
                    .rearrange("p (t e) -> p t e", t=TPC).unsqueeze(2) \
                    .to_broadcast((P, TPC, 4, 12))
                # T = S0 + fr*(S1-S0) = S0 + nf*(S0-S1)
                # Floored past the coords blocks (~28us) so the scheduler
                # cannot slot combine ops ahead of the gather-critical
                # coords chain on the in-order Vector pipe.
                D1v = D1[:, 0:TPC * 60].rearrange("p (t d e) -> p t d e", t=TPC, d=5)
                with tc.tile_wait_until(0.028 + 0.002 * k):
                    nc.vector.tensor_sub(D1v, S0, S1)
                M1v = M1[:, 0:TPC * 60].rearrange("p (t d e) -> p t d e", t=TPC, d=5)
                nc.vector.tensor_mul(M1v, D1v, frb)
                Tv = T[:, 0:TPC * 60].rearrange("p (t d e) -> p t d e", t=TPC, d=5)
                nc.vector.tensor_add(Tv, M1v, S0)
                T0v = Tv[:, :, 0:4, :]
                T1v = Tv[:, :, 1:5, :]
                D2v = D2[:, 0:TPC * 48].rearrange("p (t d e) -> p t d e", t=TPC, d=4)
                nc.vector.tensor_sub(D2v, T0v, T1v)
                M2v = M2[:, 0:TPC * 48].rearrange("p (t d e) -> p t d e", t=TPC, d=4)
                nc.vector.tensor_mul(M2v, D2v, fcb)
                Uv = U[:, 0:TPC * 48]
                nc.vector.tensor_add(
                    Uv.rearrange("p (t d e) -> p t d e", t=TPC, d=4), M2v, T0v)
                # classifier: both classes in one mul + one 2-lane reduce
                W2v = W2sb[:].rearrange("p (c f) -> p c f", c=NCLS) \
                    [:, :, tbase * 48:(tbase + TPC) * 48]
                Pm2 = Pm[:, 0:NCLS * TPC * 48] \
                    .rearrange("p (c f) -> p c f", c=NCLS)
                nc.vector.tensor_mul(
                    Pm2, Uv.unsqueeze(1).to_broadcast((P, NCLS, TPC * 48)),
                    W2v)
                nc.vector.reduce_sum(
                    r2all[:, k * NCLS:(k + 1) * NCLS].unsqueeze(2),
                    Pm2, axis=AX.X)
                tbase += TPC

            # ---- final: sum group partials, partition-reduce, bias, store
            r2 = pool.tile([P, NCLS], F32)
            r2v = r2all[:].rearrange("p (k c) -> p k c", k=NG)
            nc.vector.reduce_sum(r2[:].unsqueeze(1),
                                 r2v.rearrange("p k c -> p c k"), axis=AX.X)
            osum = ppool.tile([1, NCLS], F32)
            nc.tensor.matmul(out=osum[:], lhsT=ones[:], rhs=r2[:],
                             start=True, stop=True)
            ofin = pool.tile([1, NCLS], F32)
            nc.vector.tensor_add(ofin[:], osum[:], b2sb[:])
            nc.sync.dma_start(out, ofin[:])

    nc.compile()
    return nc


@functools.lru_cache(maxsize=2)
def _compiled(num_devices: int, svh: int, svw: int):
    return build_program(num_devices, svh, svw)


def cell_layout(img: np.ndarray) -> np.ndarray:
    """[2048, 2048, 3] f32 -> [2044*2048, 16] bf16 cell table."""
    sw = np.lib.stride_tricks.sliding_window_view(img, 5, axis=0)  # [2044,2048,3,5]
    cells = sw.transpose(0, 1, 3, 2).reshape(R, W, 15)             # (row, ch)
    buf = np.zeros((R, W, CELL), dtype=NP_BF16)
    buf[:, :, :15] = cells.astype(NP_BF16)
    return buf.reshape(R * W, CELL)


def permute_w2(W2: np.ndarray) -> np.ndarray:
    """(n, i, j, c) -> (n, j, i, c), then [p, (cls, t*48+x)] bf16."""
    w = W2.reshape(NCLS, N, PS, PS, 3).transpose(0, 1, 3, 2, 4)
    w = w.reshape(NCLS, P, TPP * 48).transpose(1, 0, 2)
    return np.ascontiguousarray(w.reshape(P, NCLS * 1536)).astype(NP_BF16)


def select_tv(tv: np.ndarray) -> np.ndarray:
    """[3,64,64] -> [3, 512]: rows {7,8},{23,24},{39,40},{55,56}."""
    sel = tv[:, (7, 8, 23, 24, 39, 40, 55, 56), :]
    return np.ascontiguousarray(sel.reshape(3, 512))


def fold_w1(W1: np.ndarray, b1: np.ndarray) -> np.ndarray:
    """[8192, 48] + [8192] -> [128, 64*49] with x0.25 and b1 as col 48."""
    w = np.empty((N * 2, 49), dtype=np.float32)
    w[:, :48] = 0.25 * np.asarray(W1, np.float32)
    w[:, 48] = np.asarray(b1, np.float32)
    return np.ascontiguousarray(w.reshape(P, 64 * 49))


def make_in_maps(topview, search_views, W1, b1, W2, b2):
    W1k = fold_w1(W1, b1)
    W2k = permute_w2(np.ascontiguousarray(W2, np.float32))
    b2k = np.ascontiguousarray(np.asarray(b2, np.float32).reshape(1, NCLS))
    mskk = np.zeros((3, 48), dtype=np.float32)
    for c in range(3):
        mskk[c, 16 * c:16 * c + 16] = 1.0
    return [{
        "tvs": select_tv(np.ascontiguousarray(topview[i], np.float32)),
        "svc": cell_layout(np.ascontiguousarray(search_views[i], np.float32)),
        "W1k": W1k, "W2k": W2k, "b2k": b2k, "msk": mskk,
    } for i in range(topview.shape[0])]


def kernel(topview, search_views, W1, b1, W2, b2, svh, svw):
    svh, svw = int(svh), int(svw)
    nc = _compiled(B, svh, svw)
    in_maps = make_in_maps(topview, search_views, W1, b1, W2, b2)
    res = bass_utils.run_bass_kernel_spmd(nc, in_maps, core_ids=list(range(B)))
    return np.concatenate([res.results[i]["out"] for i in range(B)], axis=0)


# revision 41
# speedup vs baseline: 1.0259x; 1.0259x over previous
"""Trainium2 Bass kernel for nn_DZSpecimenClfToy (v6).

Reference computation (per batch item b, B=8, one NeuronCore each):
  1. tv = bilinear_resize(topview[b], (3,64,64) -> (3,4,4))
  2. coords = sigmoid(tv.flat @ W1.T + b1).reshape(N,2)       # N=4096
  3. tl = coords*2043; 5x5x3 bilinear support per patch
  4. out[b] = bilinear_crops.flat @ W2.T + b2                 # [2]

Sharding: data-parallel over batch across 8 cores; weights replicated.

Host re-lays the search view as a cell table svc[r*2048+c] = 16 bf16
(rows r..r+4 of column c, 15 values + pad), so a patch at (r0,c0) is ONE
contiguous 79-bf16 run at cell index r0*2048+c0 (< 2^23: float magic
rounding gives the exact int index; no div/mod needed).

The HW indirect DMA supports one offset per partition per instruction
(verified: multi-offset tables generate garbage descriptors), so the
gather is 32 x [128 offsets] instructions serialized on the GpSimd Q7
(~1.4us each) - the dominant wall.  v6 minimizes everything around it
(~79.7us -> ~69.5us):

 - group 0 is small (6 patches/partition) and its coords chain runs with
   nothing fat interleaved, so the first gather issues ~14.8us instead
   of ~22us; the remaining 26 patch columns' coords run in two blocks
   floor-scheduled at 16.5/21us, and every combine group is floored past
   ~28us - without the floors the static scheduler slots fat coord muls
   or DMA-sem-waiting combine ops ahead of ready ops on the in-order
   Vector pipe, stalling the gather-critical chain (measured +4..11us).
 - b1 is folded into W1 as a 49th column (flat gets a constant 1.0).
 - the 64x64->4x4 resize runs on 3 partitions (one per channel) with a
   mask-built block-diagonal, then a 3-partition PE f32 matmul
   broadcasts flat (+the 1.0 lane) to all partitions.
 - W1 is loaded in three slices across both DMA rings so every coords
   block's slice is resident by ~10.5us.
 - group sizes descend [6,12,8,4,2]: the last gather's combine is tiny,
   so the post-gather tail is ~7us instead of ~12us.

Known dead ends (measured on HW): tensor_tensor_reduce hangs the device;
InstDMAGatherAnt (one instruction per 4096 descriptors, ~2.4us) works
but its int16 indices cannot address the 4.19M-cell table; gpsimd
tensor_scalar ops fail walrus ("engine check failed (Pool)").

floor() is computed as round_half_even(x - 0.5) in one fused op; the
half-integer cases land on a neighbouring cell with fraction 0/1, which
bilinear interpolation maps to the same value.  Fractions are produced
NEGATED (one fused op) and the combine uses S0-S1 / T0-T1 differences
to compensate.

Precision: coords matvec in f32 (bf16 shifts patches ~0.1px: fails);
patch data / combine / W2 in bf16 (~5e-3 rel err vs 2e-2 budget).
"""
import functools
from contextlib import ExitStack

import numpy as np
import ml_dtypes

import concourse.bass as bass
import concourse.tile as tile
from concourse import bacc, mybir
import concourse.bass_utils as bass_utils
from concourse.bass import IndirectOffsetOnAxis

F32 = mybir.dt.float32
BF16 = mybir.dt.bfloat16
I32 = mybir.dt.int32
ALU = mybir.AluOpType
ACT = mybir.ActivationFunctionType
AX = mybir.AxisListType

B = 8          # batch == number of cores
H = W = 2048   # search view height/width
N = 4096       # patches per item
PS = 4         # patch size
NCLS = 2       # classes
P = 128        # partitions
TPP = N // P   # patches per partition = 32

R = H - PS     # 2044 rows of 5-row cells (r0 in [0, 2043])
CELL = 16      # bf16 per (row, col) cell: 5 rows x 3 ch + 1 pad
SEG = 79       # gathered bf16 per patch: max offset 4*16+4*3+2 = 78
SEGP = 80      # SBUF stride per patch segment
MAGIC = 8388608.0      # 2**23
MAGICH = MAGIC - 0.5              # exact in f32 (below 2^23)
BIASF = MAGIC + 2.0               # r0b = BIASF + floor(tl)
C1 = BIASF * float(W)             # 2^34 + 2^12, exact in f32
NP_BF16 = ml_dtypes.bfloat16

CHUNKS = [6, 12, 8, 4, 2]  # patches per partition per gather/combine group
assert sum(CHUNKS) == TPP
NG = len(CHUNKS)
T0 = CHUNKS[0]
COORDS_B1 = 12             # second coords block size (g0 is the first)


def build_program(num_devices: int, svh: int, svw: int):
    pad = float(svh - 1 - PS)  # 2043
    assert svh == H and svw == W, (svh, svw)

    nc = bacc.Bacc("TRN2", target_bir_lowering=False, debug=False,
                   enable_asserts=False, num_devices=num_devices,
                   enable_partition_id=False)

    tvs = nc.dram_tensor("tvs", [3, 512], F32, kind="ExternalInput").ap()
    msk = nc.dram_tensor("msk", [3, 48], F32, kind="ExternalInput").ap()
    svc = nc.dram_tensor("svc", [R * W, CELL], BF16, kind="ExternalInput").ap()
    w1 = nc.dram_tensor("W1k", [P, 64 * 49], F32, kind="ExternalInput").ap()
    w2 = nc.dram_tensor("W2k", [P, NCLS * 1536], BF16, kind="ExternalInput").ap()
    b2 = nc.dram_tensor("b2k", [1, NCLS], F32, kind="ExternalInput").ap()
    out = nc.dram_tensor("out", [1, NCLS], F32, kind="ExternalOutput").ap()

    with tile.TileContext(nc) as tc:
        with ExitStack() as ctx:
            pool = ctx.enter_context(tc.tile_pool(name="main", bufs=1))
            ppool = ctx.enter_context(tc.tile_pool(name="ps", bufs=1,
                                                   space="PSUM"))

            # Everything through group 0's gathers at priority 0 so the
            # Tile scheduler keeps the head chain tight.
            prio = tc.high_priority()
            prio.__enter__()

            # ---- input DMAs.  scalar ring: tvs (gates the head), W2
            # (bulky, needed ~17us in), b2.  sync ring: W1 in two slices
            # (group-0 lanes first).
            Asb = pool.tile([3, 512], F32)
            nc.scalar.dma_start(Asb[:], tvs)
            msksb = pool.tile([3, 48], F32)
            nc.sync.dma_start(msksb[:], msk)
            # W1 split across both rings so every coords block's slice is
            # resident by ~10.5us (a late slice makes the scheduler park a
            # DMA-wait ahead of ready ops on the in-order Vector pipe).
            W1sb = pool.tile([P, 64 * 49], F32)
            ja, jb = 2 * T0 * 49, 2 * (T0 + COORDS_B1) * 49
            nc.sync.dma_start(W1sb[:, 0:ja], w1[:, 0:ja])
            nc.scalar.dma_start(W1sb[:, ja:jb], w1[:, ja:jb])
            nc.sync.dma_start(W1sb[:, jb:], w1[:, jb:])
            W2sb = pool.tile([P, NCLS * 1536], BF16)
            nc.scalar.dma_start(W2sb[:], w2)
            b2sb = pool.tile([1, NCLS], F32)
            nc.scalar.dma_start(b2sb[:], b2)

            # ---- topview 64x64 -> 4x4 resize on 3 partitions (1/channel).
            # Host pre-picked the 8 rows {16i+7, 16i+8}; row pairs then col
            # pairs {16j+7, 16j+8} are averaged (x0.25 folded into W1k).
            A4 = Asb[:].rearrange("p (i pair w) -> p i pair w", i=4, pair=2)
            V = pool.tile([3, 256], F32)
            nc.vector.tensor_add(V[:].rearrange("p (i w) -> p i w", i=4),
                                 A4[:, :, 0, :], A4[:, :, 1, :])
            V4 = V[:].rearrange("p (i j s) -> p i j s", i=4, j=4)
            flat3 = pool.tile([3, 16], F32)
            nc.vector.tensor_add(flat3[:].rearrange("p (i j) -> p i j", i=4),
                                 V4[:, :, :, 7], V4[:, :, :, 8])
            # block-diagonal [3, 49] (channel c in cols 16c..16c+15, col 48
            # = 1.0 for the folded b1), then PE broadcast to all partitions.
            bd = pool.tile([3, 49], F32)
            nc.vector.tensor_mul(
                bd[:, 0:48].rearrange("p (g e) -> p g e", g=3),
                flat3[:].unsqueeze(1).to_broadcast((3, 3, 16)),
                msksb[:].rearrange("p (g e) -> p g e", g=3))
            nc.vector.memset(bd[:, 48:49], 0.0)
            nc.vector.memset(bd[0:1, 48:49], 1.0)
            ones3 = pool.tile([3, P], F32)
            nc.vector.memset(ones3[:], 1.0)
            flatb = ppool.tile([P, 49], F32)
            nc.tensor.matmul(out=flatb[:], lhsT=ones3[:], rhs=bd[:],
                             start=True, stop=True)

            ones = pool.tile([P, 1], F32)
            nc.vector.memset(ones[:], 1.0)

            # ---- coords + gather indices --------------------------------
            mul1 = pool.tile([P, 64 * 49], F32)
            pre = pool.tile([P, 64], F32)
            sg = pool.tile([P, 64], F32)
            tl = pool.tile([P, 64], F32)
            r0b = pool.tile([P, 64], F32)   # 2^23 + floor(tl)
            nf = pool.tile([P, 64], F32)    # -fraction
            t1 = pool.tile([P, TPP], F32)
            idxm = pool.tile([P, TPP], F32)
            idxi = pool.tile([P, TPP], I32)
            frx = pool.tile([P, TPP * 12], BF16)   # -fr expanded x12
            fcx = pool.tile([P, TPP * 12], BF16)   # -fc expanded x12

            def coords_block(tbase, TPC, floor_ms=None):
                """matvec+sigmoid+index math for patch cols [tbase, tbase+TPC).

                floor_ms pins this block's matvec past group 0's chain so
                the scheduler cannot slot it ahead on the in-order Vector
                pipe (the rest of the block follows in program order).
                """
                E = nc.vector
                JC = 2 * TPC
                js = slice(2 * tbase, 2 * tbase + JC)
                ts = slice(tbase, tbase + TPC)
                m1v = mul1[:, 2 * tbase * 49:(2 * tbase + JC) * 49] \
                    .rearrange("p (j c) -> p j c", j=JC)
                with tc.tile_wait_until(floor_ms or 0, enable=floor_ms is not None):
                    nc.vector.tensor_mul(
                        m1v, W1sb[:, 2 * tbase * 49:(2 * tbase + JC) * 49]
                        .rearrange("p (j c) -> p j c", j=JC),
                        flatb[:].unsqueeze(1).to_broadcast((P, JC, 49)))
                nc.vector.reduce_sum(pre[:, js].unsqueeze(2), m1v, axis=AX.X)
                nc.scalar.activation(sg[:, js], pre[:, js], ACT.Sigmoid)
                E.tensor_scalar(tl[:, js], sg[:, js], pad, 2.0,
                                op0=ALU.mult, op1=ALU.add)
                E.tensor_scalar_add(r0b[:, js], tl[:, js], MAGICH)
                E.scalar_tensor_tensor(nf[:, js], r0b[:, js], MAGIC,
                                       tl[:, js],
                                       op0=ALU.subtract,
                                       op1=ALU.subtract)
                r0v = r0b[:, js].rearrange("p (t two) -> p t two", two=2)
                E.tensor_scalar(t1[:, ts], r0v[:, :, 0], float(W),
                                -C1, op0=ALU.mult, op1=ALU.add)
                E.scalar_tensor_tensor(idxm[:, ts], r0v[:, :, 1],
                                       2.0, t1[:, ts],
                                       op0=ALU.subtract, op1=ALU.add)
                E.tensor_single_scalar(idxi[:, ts],
                                       idxm[:, ts].bitcast(I32),
                                       0x007FFFFF, op=ALU.bitwise_and)
                nfv = nf[:, js].rearrange("p (t two) -> p t two", two=2)
                nc.vector.tensor_scalar_mul(
                    frx[:, tbase * 12:(tbase + TPC) * 12]
                    .rearrange("p (t e) -> p t e", t=TPC),
                    nfv[:, :, 0:1].to_broadcast((P, TPC, 12)), 1.0)
                nc.vector.tensor_scalar_mul(
                    fcx[:, tbase * 12:(tbase + TPC) * 12]
                    .rearrange("p (t e) -> p t e", t=TPC),
                    nfv[:, :, 1:2].to_broadcast((P, TPC, 12)), 1.0)

            Schunks = []
            for k, TPC in enumerate(CHUNKS):
                Sk = pool.tile([P, TPC * SEGP], BF16, tag=f"S{k}")
                Schunks.append(Sk)

            # Gather-critical chain at priority 0 in emission order:
            # g0 coords (small), g0 gathers, then the remaining coords in
            # two blocks, then the remaining gathers.  Combines (normal
            # priority) stream under the 45us gather phase.
            coords_block(0, T0)
            for t in range(T0):
                nc.gpsimd.indirect_dma_start(
                    out=Schunks[0][:, t * SEGP:t * SEGP + SEG],
                    out_offset=None,
                    in_=svc,
                    in_offset=IndirectOffsetOnAxis(
                        ap=idxi[:, t:t + 1], axis=0),
                )
            coords_block(T0, COORDS_B1, floor_ms=0.0165)
            coords_block(T0 + COORDS_B1, TPP - T0 - COORDS_B1, floor_ms=0.021)

            tbase = T0
            for k, TPC in enumerate(CHUNKS[1:], start=1):
                S = Schunks[k]
                for t in range(TPC):
                    tg = tbase + t
                    nc.gpsimd.indirect_dma_start(
                        out=S[:, t * SEGP:t * SEGP + SEG],
                        out_offset=None,
                        in_=svc,
                        in_offset=IndirectOffsetOnAxis(
                            ap=idxi[:, tg:tg + 1], axis=0),
                    )
                tbase += TPC
            prio.__exit__(None, None, None)

            # ---- per-group combine + classifier (overlaps later gathers)
            TPCmax = max(CHUNKS)
            D1 = pool.tile([P, TPCmax * 60], BF16)
            M1 = pool.tile([P, TPCmax * 60], BF16)
            T = pool.tile([P, TPCmax * 60], BF16)
            D2 = pool.tile([P, TPCmax * 48], BF16)
            M2 = pool.tile([P, TPCmax * 48], BF16)
            U = pool.tile([P, TPCmax * 48], BF16)
            Pm = pool.tile([P, TPCmax * 48], BF16)
            r2all = pool.tile([P, NCLS * NG], F32)

            tbase = 0
            for k, TPC in enumerate(CHUNKS):
                S = Schunks[k]
                Sc = S[:].rearrange("p (t d e) -> p t d e", t=TPC, d=5)
                S0 = Sc[:, :, :, 0:12]
                S1 = Sc[:, :, :, 3:15]
                frb = frx[:, tbase * 12:(tbase + TPC) * 12] \
                    .rearrange("p (t e) -> p t e", t=TPC).unsqueeze(2) \
                    .to_broadcast((P, TPC, 5, 12))
                fcb = fcx[:, tbase * 12:(tbase + TPC) * 12] \
                    .rearrange("p (t e) -> p t e", t=TPC).unsqueeze(2) \
                    .to_broadcast((P, TPC, 4, 12))
                # T = S0 + fr*(S1-S0) = S0 + nf*(S0-S1)
                # Floored past the coords blocks (~28us) so the scheduler
                # cannot slot combine ops ahead of the gather-critical
                # coords chain on the in-order Vector pipe.
                D1v = D1[:, 0:TPC * 60].rearrange("p (t d e) -> p t d e", t=TPC, d=5)
                with tc.tile_wait_until(0.028 + 0.002 * k):
                    nc.vector.tensor_sub(D1v, S0, S1)
                M1v = M1[:, 0:TPC * 60].rearrange("p (t d e) -> p t d e", t=TPC, d=5)
                nc.vector.tensor_mul(M1v, D1v, frb)
                Tv = T[:, 0:TPC * 60].rearrange("p (t d e) -> p t d e", t=TPC, d=5)
                nc.vector.tensor_add(Tv, M1v, S0)
                T0v = Tv[:, :, 0:4, :]
                T1v = Tv[:, :, 1:5, :]
                D2v = D2[:, 0:TPC * 48].rearrange("p (t d e) -> p t d e", t=TPC, d=4)
                nc.vector.tensor_sub(D2v, T0v, T1v)
                M2v = M2[:, 0:TPC * 48].rearrange("p (t d e) -> p t d e", t=TPC, d=4)
                nc.vector.tensor_mul(M2v, D2v, fcb)
                Uv = U[:, 0:TPC * 48]
                nc.vector.tensor_add(
                    Uv.rearrange("p (t d e) -> p t d e", t=TPC, d=4), M2v, T0v)
                for c in range(NCLS):
                    nc.vector.tensor_mul(
                        Pm[:, 0:TPC * 48], Uv,
                        W2sb[:, c * 1536 + tbase * 48:
                             c * 1536 + (tbase + TPC) * 48])
                    nc.vector.reduce_sum(
                        r2all[:, k * NCLS + c:k * NCLS + c + 1].unsqueeze(2),
                        Pm[:, 0:TPC * 48].unsqueeze(1), axis=AX.X)
                tbase += TPC

            # ---- final: sum group partials, partition-reduce, bias, store
            r2 = pool.tile([P, NCLS], F32)
            r2v = r2all[:].rearrange("p (k c) -> p k c", k=NG)
            nc.vector.reduce_sum(r2[:].unsqueeze(1),
                                 r2v.rearrange("p k c -> p c k"), axis=AX.X)
            osum = ppool.tile([1, NCLS], F32)
            nc.tensor.matmul(out=osum[:], lhsT=ones[:], rhs=r2[:],
                             start=True, stop=True)
            ofin = pool.tile([1, NCLS], F32)
            nc.vector.tensor_add(ofin[:], osum[:], b2sb[:])
            nc.sync.dma_start(out, ofin[:])

    nc.compile()
    return nc


@functools.lru_cache(maxsize=2)
def _compiled(num_devices: int, svh: int, svw: int):
    return build_program(num_devices, svh, svw)


def cell_layout(img: np.ndarray) -> np.ndarray:
    """[2048, 2048, 3] f32 -> [2044*2048, 16] bf16 cell table."""
    sw = np.lib.stride_tricks.sliding_window_view(img, 5, axis=0)  # [2044,2048,3,5]
    cells = sw.transpose(0, 1, 3, 2).reshape(R, W, 15)             # (row, ch)
    buf = np.zeros((R, W, CELL), dtype=NP_BF16)
    buf[:, :, :15] = cells.astype(NP_BF16)
    return buf.reshape(R * W, CELL)


def permute_w2(W2: np.ndarray) -> np.ndarray:
    """(n, i, j, c) -> (n, j, i, c), then [p, (cls, t*48+x)] bf16."""
    w = W2.reshape(NCLS, N, PS, PS, 3).transpose(0, 1, 3, 2, 4)
    w = w.reshape(NCLS, P, TPP * 48).transpose(1, 0, 2)
    return np.ascontiguousarray(w.reshape(P, NCLS * 1536)).astype(NP_BF16)


def select_tv(tv: np.ndarray) -> np.ndarray:
    """[3,64,64] -> [3, 512]: rows {7,8},{23,24},{39,40},{55,56}."""
    sel = tv[:, (7, 8, 23, 24, 39, 40, 55, 56), :]
    return np.ascontiguousarray(sel.reshape(3, 512))


def fold_w1(W1: np.ndarray, b1: np.ndarray) -> np.ndarray:
    """[8192, 48] + [8192] -> [128, 64*49] with x0.25 and b1 as col 48."""
    w = np.empty((N * 2, 49), dtype=np.float32)
    w[:, :48] = 0.25 * np.asarray(W1, np.float32)
    w[:, 48] = np.asarray(b1, np.float32)
    return np.ascontiguousarray(w.reshape(P, 64 * 49))


def make_in_maps(topview, search_views, W1, b1, W2, b2):
    W1k = fold_w1(W1, b1)
    W2k = permute_w2(np.ascontiguousarray(W2, np.float32))
    b2k = np.ascontiguousarray(np.asarray(b2, np.float32).reshape(1, NCLS))
    mskk = np.zeros((3, 48), dtype=np.float32)
    for c in range(3):
        mskk[c, 16 * c:16 * c + 16] = 1.0
    return [{
        "tvs": select_tv(np.ascontiguousarray(topview[i], np.float32)),
        "svc": cell_layout(np.ascontiguousarray(search_views[i], np.float32)),
        "W1k": W1k, "W2k": W2k, "b2k": b2k, "msk": mskk,
    } for i in range(topview.shape[0])]


def kernel(topview, search_views, W1, b1, W2, b2, svh, svw):
    svh, svw = int(svh), int(svw)
    nc = _compiled(B, svh, svw)
    in_maps = make_in_maps(topview, search_views, W1, b1, W2, b2)
    res = bass_utils.run_bass_kernel_spmd(nc, in_maps, core_ids=list(range(B)))
    return np.concatenate([res.results[i]["out"] for i in range(B)], axis=0)
